# revision 65
# baseline (speedup 1.0000x reference)
"""Trainium2 Bass kernel for a dense transformer block (B=2, T=2048, C=1024,
H=16, Dff=4096), SPMD across 8 NeuronCores.

Sharding: attention is head-parallel (2 heads/core); one AllToAll per batch
redistributes the attention output into a token-parallel layout; projection,
layernorms and the FFN then run on each core's 512-token slice with full
weights. All on-device activations are kept feature-major (transposed) so
every matmul consumes weights exactly as stored; the host performs the
x -> x^T and out^T -> out transposes during input/output marshalling.
Matmuls run in float32r (fp32 storage, FP22 multiply, fp32 accumulate).

Phases 2+3 (proj+LN1+FFN+LN2) are emitted per 256-token batch half: the
batch-0 half only depends on the first AllToAll, so its whole chain
executes while the second AllToAll is still in flight.
"""

import sys

sys.path.insert(0, "/opt/trn_rl_repo")

import numpy as np
import ml_dtypes
import concourse.bacc as bacc
import concourse.mybir as mybir
import concourse.tile as tile
import concourse.bass_utils as bass_utils

try:  # make the NTFF profile shim importable as antenv.axon_hooks
    import types

    import antenv

    if "antenv.axon_hooks" not in sys.modules:
        _ah = types.ModuleType("antenv.axon_hooks")
        _ah._hook = None

        def _set_hook(h):
            _ah._hook = h

        def _get_hook():
            return _ah._hook

        _ah.set_axon_ntff_profile_hook = _set_hook
        _ah.get_axon_ntff_profile_hook = _get_hook
        sys.modules["antenv.axon_hooks"] = _ah
        antenv.axon_hooks = _ah
        try:
            from trn_agent_boot.trn_boot import _ntff_profile_via_ctypes

            _set_hook(_ntff_profile_via_ctypes("/opt/axon/libaxon_pjrt.so"))
        except Exception:
            pass
except Exception:
    pass

f32 = mybir.dt.float32
f32r = mybir.dt.float32r
bf16 = mybir.dt.bfloat16
fp8 = mybir.dt.float8e4
AF = mybir.ActivationFunctionType
ALU = mybir.AluOpType
DR = mybir.MatmulPerfMode.DoubleRow

WS = 64.0             # fp8 weight pre-scale (keeps weights out of subnormals)
ESC = 0.125 / (WS * WS)  # exp scale absorbing the q,k weight scales

NC = 8          # cores
B = 2           # batch
T = 2048        # sequence length
C = 1024        # model dim
H = 16          # heads
HD = 64         # head dim
HPC = H // NC   # heads per core (2)
DH = HPC * HD   # per-core head cols (128)
DFF = 4096
TOK = B * T     # 4096 tokens
TOKC = TOK // NC  # 512 tokens per core
CT = C // 128   # 8 c-tiles
FT = DFF // 128  # 32 ff-tiles
KT = T // 128   # 16 k-tiles per batch
QC = T // 512   # 4 q-chunks of 512 per batch
HT = TOKC // B  # 256 tokens per batch per core
LN_EPS = 1e-5

_CACHE = {}


def _build(debug=False):
    nc = bacc.Bacc("TRN2", target_bir_lowering=False, debug=False, num_devices=NC)

    # ---- DRAM I/O (per-core values supplied via in_maps) ----
    xt_d = nc.dram_tensor("xt8", [TOK // 512, 128, CT, 512], fp8,
                          kind="ExternalInput")
    wq_d = nc.dram_tensor("wq_c", [128, CT, 128], fp8, kind="ExternalInput")
    wk_d = nc.dram_tensor("wk_c", [128, CT, 128], fp8, kind="ExternalInput")
    wv_d = nc.dram_tensor("wv_c", [128, CT, 128], fp8, kind="ExternalInput")
    xres_d = nc.dram_tensor("xres_c", [C, TOKC], f32r, kind="ExternalInput")
    wp_d = nc.dram_tensor("wp8", [128, CT, C], fp8, kind="ExternalInput")
    w1_d = nc.dram_tensor("w1p", [FT, 128, CT, 128], bf16, kind="ExternalInput")
    w2_d = nc.dram_tensor("w2", [DFF, C], bf16, kind="ExternalInput")
    bias_d = nc.dram_tensor("biaspack", [128, 6 * CT], f32, kind="ExternalInput")
    b1_d = nc.dram_tensor("b1t", [128, FT], f32, kind="ExternalInput")
    ones_d = nc.dram_tensor("onesp", [128, 128], f32r, kind="ExternalInput")
    ident_d = nc.dram_tensor("ident", [128, 128], f32, kind="ExternalInput")
    mask_d = nc.dram_tensor("cmask", [128, 4, 2, 512], bf16, kind="ExternalInput")
    out_d = nc.dram_tensor("out", [C, TOKC], f32, kind="ExternalOutput")
    if debug:
        dbg_attn = nc.dram_tensor("dbg_attn", [2, HD, TOK], f32, kind="ExternalOutput")
        dbg_qkv = nc.dram_tensor("dbg_qkv", [3, DH, TOK], f32, kind="ExternalOutput")

    with tile.TileContext(nc) as tc:
        with (
            nc.allow_low_precision(reason="float32r matmul inputs (~6e-5 rounding)"),
            tc.tile_pool(name="const", bufs=1) as p_const,
            tc.tile_pool(name="ln1p", bufs=CT) as p_ln1,
            tc.tile_pool(name="dram", bufs=1, space="DRAM") as p_dram,
        ):
            # ---- persistent constants (scalar DMA queue: keeps the sync
            # queue free for the critical-path QKV weight + x loads) ----
            ones = p_const.tile([128, 128], f32r, tag="ones")
            nc.scalar.dma_start(ones[:], ones_d[:])
            onesb = p_const.tile([128, 128], bf16, tag="onesb")
            nc.vector.tensor_copy(onesb[:], ones[:].bitcast(f32))
            biasp = p_const.tile([128, 6 * CT], f32, tag="biasp")
            nc.scalar.dma_start(biasp[:], bias_d[:])
            b1t = p_const.tile([128, FT], f32, tag="b1t")
            nc.scalar.dma_start(b1t[:], b1_d[:])
            # bias pack columns: [bproj | b2 | g1 | be1 | g2 | be2]
            bproj_b = biasp[:, 0 * CT:1 * CT]
            b2_b = biasp[:, 1 * CT:2 * CT]
            g1_b = biasp[:, 2 * CT:3 * CT]
            be1_b = biasp[:, 3 * CT:4 * CT]
            g2_b = biasp[:, 4 * CT:5 * CT]
            be2_b = biasp[:, 5 * CT:6 * CT]

            # two half-AllToAlls: batch-0 shards exchange while batch-1
            # attention still computes. Core c owns tokens
            # [c*256,(c+1)*256) of each batch (512 total).
            a2a_in = [
                p_dram.tile([NC, DH, HT], fp8, tag=f"a2ai{b}", name=f"a2ai{b}")
                for b in range(B)
            ]
            a2a_out = [
                p_dram.tile([NC, DH, HT], fp8, tag=f"a2ao{b}", name=f"a2ao{b}")
                for b in range(B)
            ]

            def layer_norm(x_tiles, cols, out_slice, g_b, be_b, sum_lhsT,
                           sq_dtype, tmp_pool, ps_pool, ps_tag="ps",
                           ps_bufs=8, post_ct=None):
                """Feature-major LN on token columns `cols` of CT tiles."""
                nh = cols.stop - cols.start
                s1 = ps_pool.tile([1, nh], f32, tag=ps_tag, bufs=ps_bufs)
                s2 = ps_pool.tile([1, nh], f32, tag=ps_tag, bufs=ps_bufs)
                for ct in range(CT):
                    nc.tensor.matmul(
                        s1[:], sum_lhsT[:, 0:1], x_tiles[ct][:, cols],
                        start=(ct == 0), stop=(ct == CT - 1),
                    )
                for ct in range(CT):
                    sq = tmp_pool.tile([128, nh], sq_dtype, tag=f"sq{nh}")
                    if sq_dtype == bf16:
                        nc.vector.tensor_mul(
                            sq[:], x_tiles[ct][:, cols],
                            x_tiles[ct][:, cols],
                        )
                    else:
                        nc.vector.tensor_mul(
                            sq[:],
                            x_tiles[ct][:, cols].bitcast(f32),
                            x_tiles[ct][:, cols].bitcast(f32),
                        )
                    nc.tensor.matmul(
                        s2[:], sum_lhsT[:, 0:1], sq[:],
                        start=(ct == 0), stop=(ct == CT - 1),
                    )
                nmu = tmp_pool.tile([1, nh], f32r, tag=f"nmu{nh}")
                nc.vector.tensor_scalar_mul(nmu[:], s1[:], -1.0 / C)
                ex2 = tmp_pool.tile([1, nh], f32, tag=f"ex2{nh}")
                nc.vector.tensor_scalar_mul(ex2[:], s2[:], 1.0 / C)
                mu2 = tmp_pool.tile([1, nh], f32, tag=f"mu2{nh}")
                nc.vector.tensor_mul(
                    mu2[:], nmu[:].bitcast(f32),
                    nmu[:].bitcast(f32),
                )
                var = tmp_pool.tile([1, nh], f32, tag=f"var{nh}")
                nc.vector.tensor_sub(var[:], ex2[:], mu2[:])
                nc.vector.tensor_scalar_add(var[:], var[:], LN_EPS)
                sd = tmp_pool.tile([1, nh], f32r, tag=f"sd{nh}")
                nc.scalar.activation(sd[:], var[:], AF.Sqrt, bias=0.0)
                bmu = ps_pool.tile([128, nh], f32, tag=ps_tag, bufs=ps_bufs)
                nc.tensor.matmul(
                    bmu[:], ones[0:1, :], nmu[:], start=True, stop=True
                )
                brs = ps_pool.tile([128, nh], f32, tag=ps_tag, bufs=ps_bufs)
                nc.tensor.matmul(
                    brs[:], ones[0:1, :], sd[:], start=True, stop=True
                )
                bmu_sb = tmp_pool.tile([128, nh], f32, tag=f"bmu{nh}")
                nc.scalar.copy(bmu_sb[:], bmu[:])
                brs_sb = tmp_pool.tile([128, nh], f32, tag=f"brs{nh}")
                nc.vector.reciprocal_approx_fast(brs_sb[:], brs[:])
                for ct in range(CT):
                    t1 = tmp_pool.tile([128, nh], f32, tag=f"lntmp{nh}")
                    nc.vector.tensor_add(
                        t1[:],
                        x_tiles[ct][:, cols] if x_tiles[ct].dtype == bf16
                        else x_tiles[ct][:, cols].bitcast(f32),
                        bmu_sb[:],
                    )
                    t2 = tmp_pool.tile([128, nh], f32, tag=f"lntmp2{nh}")
                    nc.vector.tensor_mul(t2[:], t1[:],
                                         brs_sb[:])
                    nc.scalar.activation(
                        out_slice(ct), t2[:], AF.Identity,
                        bias=be_b[:, ct:ct + 1], scale=g_b[:, ct:ct + 1],
                    )
                    if post_ct is not None:
                        post_ct(ct)

            ln1 = [
                p_ln1.tile([128, TOKC], bf16, tag="ln1", name=f"ln1_{i}")
                for i in range(CT)
            ]
            # gathered attention output [feature-of-core-i, token] per half,
            # persistent so the gathers can be issued right after each
            # AllToAll inside phase 1 (gpsimd queue, in trigger order)
            agp = [
                p_ln1.tile([128, NC, HT], fp8, tag="ag", name=f"ag{i}")
                for i in range(B)
            ]
            # proj weights + residual: loaded up front so they are not
            # queued behind the attention-phase staging DMAs
            wp_sb = p_ln1.tile([128, CT, C], fp8, tag="wp")
            nc.sync.dma_start(wp_sb[:], wp_d[:])
            xres = []
            for ct in range(CT):
                t = p_ln1.tile([128, TOKC], f32r, tag="xres")
                nc.scalar.dma_start(t[:], xres_d[ct * 128:(ct + 1) * 128, :])
                xres.append(t)

            # ======== phase 1: QKV + attention (head-parallel) ========
            with (
                tc.tile_pool(name="attn", bufs=1) as p_attn,
                tc.tile_pool(name="p1c", bufs=1) as p1c,
                tc.tile_pool(name="xt", bufs=8) as p_xt,
                tc.tile_pool(name="qkv", bufs=1) as p_qkv,
                tc.tile_pool(name="es", bufs=3) as p_es,
                tc.tile_pool(name="small", bufs=2) as p_small,
                tc.tile_pool(name="ps1", bufs=6, space="PSUM") as ps1,
            ):
                # per-head attention outputs (feature-major rows 0-63)
                attnh = [
                    p_attn.tile([HD, TOK], fp8, tag=f"attn{h}", name=f"attnh{h}")
                    for h in range(HPC)
                ]
                # QKV weights first: they gate the very first matmul
                wq_sb = p1c.tile([128, CT, 128], fp8, tag="wq")
                wk_sb = p1c.tile([128, CT, 128], fp8, tag="wk")
                wv_sb = p1c.tile([128, CT, 128], fp8, tag="wv")
                nc.sync.dma_start(wq_sb[:], wq_d[:])
                nc.sync.dma_start(wk_sb[:], wk_d[:])
                nc.sync.dma_start(wv_sb[:], wv_d[:])
                ident = p1c.tile([128, 128], f32, tag="ident")
                nc.scalar.dma_start(ident[:], ident_d[:])
                identb = p1c.tile([128, 128], bf16, tag="identb")
                nc.vector.tensor_copy(identb[:], ident[:])
                masks = p1c.tile([128, 4, 2, 512], bf16, tag="masks")
                nc.scalar.dma_start(masks[:], mask_d[:])
                # exp-scratch tiles are multiplied by 0/1 masks in regions
                # that may be stale; zero them once so 0*garbage != NaN
                eds = [
                    p_es.tile([128, 2, 512], bf16, tag="esd", bufs=2,
                              name=f"esd{i}")
                    for i in range(2)
                ]
                for t in eds:
                    nc.vector.memset(t[:], 0.0)

                for b in range(B):
                    qT = p_qkv.tile([DH, T], f32r, tag="q")
                    kT = p_qkv.tile([DH, T], f32r, tag="k")
                    vT = p_qkv.tile([DH, T], bf16, tag="v")

                    # ---- QKV projections (feature-major), fp8 DoubleRow,
                    # x^T streamed per 512-token chunk ----
                    for n in range(QC):
                        ncol = slice(n * 512, (n + 1) * 512)
                        xt_c = p_xt.tile([128, CT, 512], fp8, tag="xt", bufs=3)
                        if b == 0 and n == 0:
                            # split the very first load so the first DoubleRow
                            # matmul starts after 128KB instead of 512KB
                            for i2 in range(CT // 2):
                                pr = slice(2 * i2, 2 * i2 + 2)
                                nc.sync.dma_start(
                                    xt_c[:, pr, :], xt_d[0][:, pr, :]
                                )
                        else:
                            nc.sync.dma_start(xt_c[:], xt_d[b * QC + n])
                        pq = ps1.tile([128, 512], f32, tag="sps", bufs=3)
                        pk = ps1.tile([128, 512], f32, tag="sps", bufs=3)
                        for i2 in range(CT // 2):
                            st = i2 == 0
                            sp = i2 == CT // 2 - 1
                            pair = slice(2 * i2, 2 * i2 + 2)
                            nc.tensor.matmul(
                                pq[:], wq_sb[:, pair, :], xt_c[:, pair, :],
                                start=st, stop=sp, perf_mode=DR,
                            )
                            nc.tensor.matmul(
                                pk[:], wk_sb[:, pair, :], xt_c[:, pair, :],
                                start=st, stop=sp, perf_mode=DR,
                            )
                        nc.vector.tensor_copy(qT[:, ncol], pq[:])
                        nc.vector.tensor_copy(kT[:, ncol], pk[:])
                        pv = ps1.tile([128, 512], f32, tag="sps", bufs=3)
                        for i2 in range(CT // 2):
                            pair = slice(2 * i2, 2 * i2 + 2)
                            nc.tensor.matmul(
                                pv[:], wv_sb[:, pair, :], xt_c[:, pair, :],
                                start=(i2 == 0), stop=(i2 == CT // 2 - 1),
                                perf_mode=DR,
                            )
                        nc.vector.tensor_copy(vT[:, ncol], pv[:])

                    # ---- V -> token-major bf16, descaled by 1/WS; 80-wide
                    # per-head slots: [v(64)|ones|pad] x 2 heads ----
                    vt = p_qkv.tile([128, KT, 2, 80], bf16, tag="vt")
                    nc.vector.tensor_copy(
                        vt[:, :, :, 64:65], ones[:, 0:2 * KT]
                    )
                    for kt in range(KT):
                        pt = ps1.tile([128, 2, 64], bf16, tag="oacc", bufs=2)
                        nc.tensor.transpose(
                            pt[:], vT[:, kt * 128:(kt + 1) * 128], identb[:]
                        )
                        nc.vector.tensor_scalar_mul(
                            vt[:, kt, :, 0:64], pt[:], 1.0 / WS
                        )

                    # ---- causal attention: both heads interleaved per k-tile
                    # (adjacent score matmuls pack into disjoint array rows);
                    # exp in bf16 (raw softmax numerators overflow fp8) ----
                    for j in range(QC):
                        nkt = 4 * j + 4
                        oacc = [
                            ps1.tile([65, 512], f32, tag="oacc", bufs=2,
                                     name=f"oacc{h}")
                            for h in range(HPC)
                        ]
                        for kt in range(nkt):
                            m = kt - 4 * j
                            # both heads' scores into one 2-bank PSUM tile so a
                            # single exp (and one wait) covers both attnV MMs
                            spair = ps1.tile([128, 2, 512], f32, tag="sps",
                                             bufs=3)
                            for h in range(HPC):
                                hrow = slice(h * 64, (h + 1) * 64)
                                nc.tensor.matmul(
                                    spair[:, h, :],
                                    kT[hrow, kt * 128:(kt + 1) * 128],
                                    qT[hrow, j * 512:(j + 1) * 512],
                                    start=True, stop=True,
                                    tile_position=(64 * h, 0),
                                )
                            epair = p_es.tile([128, 2, 512], bf16, tag="es",
                                              bufs=6)
                            if m < 0:
                                nc.scalar.activation(
                                    epair[:], spair[:], AF.Exp, scale=ESC
                                )
                            else:
                                ed = eds[kt % 2]
                                # cols < 128m are masked to 0 below, so exp
                                # only the live columns
                                nc.scalar.activation(
                                    ed[:, :, 128 * m:512],
                                    spair[:, :, 128 * m:512], AF.Exp,
                                    scale=ESC,
                                )
                                nc.vector.tensor_mul(
                                    epair[:], ed[:], masks[:, m, :, :]
                                )
                            for h in range(HPC):
                                nc.tensor.matmul(
                                    oacc[h][:],
                                    vt[:, kt, h, 0:65],
                                    epair[:, h, :],
                                    start=(kt == 0), stop=(kt == nkt - 1),
                                )
                        for h in range(HPC):
                            # free the PSUM accum early via two same-base copies
                            osb = p_small.tile([64, 512], f32, tag="osb", bufs=2)
                            nc.vector.tensor_copy(osb[:], oacc[h][0:64, :])
                            sr = p_small.tile([128, 512], f32r, tag="sr")
                            nc.vector.tensor_copy(sr[64:65, :], oacc[h][64:65, :])
                            # broadcast sums across partitions, then approx-recip
                            bps = ps1.tile([64, 512], f32, tag="oacc", bufs=2)
                            nc.tensor.matmul(
                                bps[:], ones[64:65, 0:64], sr[64:65, :],
                                start=True, stop=True, tile_position=(64, 0),
                            )
                            ibc = p_small.tile([64, 512], f32, tag="ibc")
                            nc.vector.reciprocal_approx_fast(ibc[:], bps[:])
                            nc.vector.tensor_mul(
                                attnh[h][:, b * T + j * 512:b * T + (j + 1) * 512],
                                osb[:],
                                ibc[:],
                            )
                        # stage this chunk's two A2A shards immediately
                        for s in (2 * j, 2 * j + 1):
                            for h in range(HPC):
                                nc.sync.dma_start(
                                    a2a_in[b][s, h * 64:(h + 1) * 64, :],
                                    attnh[h][:, b * T + s * HT:b * T + (s + 1) * HT],
                                )

                    if debug:
                        nc.sync.dma_start(
                            dbg_qkv[0, :, b * T:(b + 1) * T], qT[:].bitcast(f32)
                        )
                        nc.sync.dma_start(
                            dbg_qkv[1, :, b * T:(b + 1) * T], kT[:].bitcast(f32)
                        )
                        nc.sync.dma_start(dbg_qkv[2, :, b * T:(b + 1) * T], qT[:].bitcast(f32))

                    # ---- AllToAll for this batch's token shards (staged
                    # incrementally above); the b=0 exchange overlaps
                    # batch-1 QKV + attention ----
                    nc.gpsimd.collective_compute(
                        "AllToAll",
                        ALU.bypass,
                        replica_groups=[list(range(NC))],
                        ins=[a2a_in[b][:].opt()],
                        outs=[a2a_out[b][:].opt()],
                    )
                    # gather immediately (gpsimd queue: ordered right behind
                    # this batch's collective, ahead of the next trigger)
                    for i in range(NC):
                        nc.gpsimd.dma_start(agp[b][:, i, :], a2a_out[b][i])

                if debug:
                    for h in range(HPC):
                        dbg_t = p_attn.tile([HD, TOK], f32, tag=f"dbgc{h}")
                        nc.vector.tensor_copy(dbg_t[:], attnh[h][:])
                        nc.sync.dma_start(dbg_attn[h], dbg_t[:])

            # ======== phases 2+3 per batch half: gather + proj + LN1 +
            # FFN + LN2 + out on 256 tokens at a time. Half 0 (batch 0)
            # only needs the first AllToAll, so its whole chain executes
            # while the second AllToAll is still in flight. ====
            with (
                tc.tile_pool(name="wbig", bufs=8) as p_wbig,
                tc.tile_pool(name="act2", bufs=8) as p_act2,
                tc.tile_pool(name="tmp2", bufs=2) as p_tmp2,
                tc.tile_pool(name="hff", bufs=FT) as p_hff,
                tc.tile_pool(name="w1s", bufs=4) as p_w1,
                tc.tile_pool(name="w2s", bufs=4) as p_w2,
                tc.tile_pool(name="ps2", bufs=8, space="PSUM") as ps2,
            ):
                x1 = [
                    p_act2.tile([128, TOKC], bf16, tag="x1", name=f"x1_{i}")
                    for i in range(CT)
                ]
                hff = [
                    p_hff.tile([128, TOKC], bf16, tag="hff", name=f"hff{i}")
                    for i in range(FT)
                ]
                # x2 = ln1 + ffn2 out reuses the xres tiles (dead once the
                # proj residual has been added into x1)
                x2 = xres

                def proj_ln1(half):
                    """Attention proj + LN1 for one batch half (the ag
                    gathers were already issued in phase 1)."""
                    cols = slice(half * HT, (half + 1) * HT)
                    # ---- proj (fp8 DoubleRow) + residual ----
                    for mt in range(CT):
                        yps = ps2.tile([128, HT], f32, tag="ps")
                        for i2 in range(CT // 2):
                            pair = slice(2 * i2, 2 * i2 + 2)
                            nc.tensor.matmul(
                                yps[:],
                                wp_sb[:, pair, mt * 128:(mt + 1) * 128],
                                agp[half][:, pair, :],
                                start=(i2 == 0), stop=(i2 == CT // 2 - 1),
                                perf_mode=DR,
                            )
                        t1 = p_tmp2.tile([128, HT], f32, tag="projt")
                        nc.scalar.activation(
                            t1[:], yps[:], AF.Identity,
                            bias=bproj_b[:, mt:mt + 1], scale=1.0 / WS,
                        )
                        nc.vector.tensor_add(
                            x1[mt][:, cols], t1[:],
                            xres[mt][:, cols].bitcast(f32),
                        )
                    layer_norm(
                        x1, cols, lambda ct: ln1[ct][:, cols],
                        g1_b, be1_b, onesb, bf16, p_tmp2, ps2,
                    )

                def ffn1_mt(mt, cols, w1t):
                    yps = ps2.tile([128, cols.stop - cols.start], f32,
                                   tag="ps", name="f1ps")
                    for kt in range(CT):
                        nc.tensor.matmul(
                            yps[:], w1t[:, kt, :], ln1[kt][:, cols],
                            start=(kt == 0), stop=(kt == CT - 1),
                        )
                    nc.scalar.activation(
                        hff[mt][:, cols], yps[:], AF.Relu,
                        bias=b1t[:, mt:mt + 1],
                    )

                # ---- half 0's proj/LN1 + the first SPLIT FFN1 tiles run
                # 256 wide while the second AllToAll is in flight; all
                # remaining FFN work runs full width ----
                SPLIT = 12
                proj_ln1(0)
                h0 = slice(0, HT)
                h1 = slice(HT, TOKC)
                full = slice(0, TOKC)
                for mt in range(SPLIT):
                    w1t = p_w1.tile([128, CT, 128], bf16, tag="w1")
                    nc.sync.dma_start(w1t[:], w1_d[mt])
                    ffn1_mt(mt, h0, w1t)
                proj_ln1(1)
                for mt in range(SPLIT, FT):
                    w1t = p_w1.tile([128, CT, 128], bf16, tag="w1")
                    nc.sync.dma_start(w1t[:], w1_d[mt])
                    ffn1_mt(mt, full, w1t)
                for mt in range(SPLIT):
                    w1t = p_w1.tile([128, CT, 128], bf16, tag="w1")
                    nc.sync.dma_start(w1t[:], w1_d[mt])
                    ffn1_mt(mt, h1, w1t)

                # ---- FFN2 (kt-outer, 8 accumulators, full width) + LN2 ----
                accs = [
                    ps2.tile([128, TOKC], f32, tag="ps", name=f"acc{mt}")
                    for mt in range(CT)
                ]
                for kt in range(FT):
                    w2t = p_w2.tile([128, C], bf16, tag="w2")
                    nc.sync.dma_start(
                        w2t[:], w2_d[kt * 128:(kt + 1) * 128, :]
                    )
                    for mt in range(CT):
                        nc.tensor.matmul(
                            accs[mt][:],
                            w2t[:, mt * 128:(mt + 1) * 128],
                            hff[kt][:],
                            start=(kt == 0), stop=(kt == FT - 1),
                        )
                for mt in range(CT):
                    t1 = p_tmp2.tile([128, TOKC], f32, tag="ffn2t")
                    nc.scalar.activation(
                        t1[:], accs[mt][:], AF.Identity,
                        bias=b2_b[:, mt:mt + 1],
                    )
                    nc.vector.tensor_add(
                        x2[mt][:], t1[:], ln1[mt][:]
                    )
                # LN2 stats are per-token, so run it as two half-width
                # passes (reusing LN1's 256-wide temp tags); the output
                # streams through a small rotating tile DMA'd per c-tile
                for half in range(B):
                    cols = slice(half * HT, (half + 1) * HT)
                    ocur = [None]

                    def out_slice2(ct):
                        ot = p_tmp2.tile([128, HT], f32, tag="outt",
                                         bufs=3, name="otile")
                        ocur[0] = ot
                        return ot[:]

                    def post_ct(ct, _cols=cols):
                        nc.sync.dma_start(
                            out_d[ct * 128:(ct + 1) * 128, _cols], ocur[0][:]
                        )

                    layer_norm(
                        x2, cols, out_slice2,
                        g2_b, be2_b, ones, f32r, p_tmp2, ps2,
                        post_ct=post_ct,
                    )

    nc.compile()
    return nc


def _fp8(a, scale=1.0):
    """Quantize to TRN e4m3 (max normal +-240)."""
    a = np.asarray(a, dtype=np.float32) * np.float32(scale)
    a = np.clip(a, -240.0, 240.0)
    return np.ascontiguousarray(a.astype(ml_dtypes.float8_e4m3))


def _pack_inputs(inputs):
    """Host-side sharding/marshalling. Returns in_maps for the 8 cores."""
    x = np.asarray(inputs["x"], dtype=np.float32)
    xf = np.ascontiguousarray(x.reshape(TOK, C))
    xt = np.ascontiguousarray(xf.T)  # [C, TOK]
    # fp8 x^T chunked [TOK/512, 128, CT, 512]
    xt8 = _fp8(
        xt.reshape(CT, 128, TOK // 512, 512).transpose(2, 1, 0, 3)
    )
    wq = np.asarray(inputs["wq"], dtype=np.float32)
    wk = np.asarray(inputs["wk"], dtype=np.float32)
    wv = np.asarray(inputs["wv"], dtype=np.float32)
    # wp8 [128, CT, C]: wp8[p, i, c] = w_proj[i*128+p, c] * WS
    wp8 = _fp8(
        np.asarray(inputs["w_proj"], dtype=np.float32)
        .reshape(CT, 128, C).transpose(1, 0, 2),
        WS,
    )
    w1 = np.asarray(inputs["w1"], dtype=np.float32)
    w2 = np.ascontiguousarray(
        np.asarray(inputs["w2"], dtype=np.float32).astype(ml_dtypes.bfloat16)
    )
    # w1 packed per ff-tile: [FT, 128(p), CT, 128(f)];  w1 is [C, DFF]
    w1p = np.ascontiguousarray(
        w1.reshape(CT, 128, FT, 128).transpose(2, 1, 0, 3).astype(ml_dtypes.bfloat16)
    )

    def tile_vec(v, n):
        return np.ascontiguousarray(
            np.asarray(v, dtype=np.float32).reshape(n, 128).T
        )

    biaspack = np.zeros((128, 6 * CT), dtype=np.float32)
    biaspack[:, 0 * CT:1 * CT] = tile_vec(inputs["b_proj"], CT)
    biaspack[:, 1 * CT:2 * CT] = tile_vec(inputs["b2"], CT)
    biaspack[:, 2 * CT:3 * CT] = tile_vec(inputs["g1"], CT)
    biaspack[:, 3 * CT:4 * CT] = tile_vec(inputs["be1"], CT)
    biaspack[:, 4 * CT:5 * CT] = tile_vec(inputs["g2"], CT)
    biaspack[:, 5 * CT:6 * CT] = tile_vec(inputs["be2"], CT)
    b1t = tile_vec(inputs["b1"], FT)

    # causal masks for the 4 diagonal offsets, duplicated across the two
    # heads: [128, 4, 2, 512]
    r = np.arange(128)[:, None]
    ccol = np.arange(512)[None, :]
    cmask = np.stack(
        [(ccol >= r + 128 * m).astype(np.float32) for m in range(4)], axis=1
    )
    cmask = np.ascontiguousarray(
        np.repeat(cmask[:, :, None, :], 2, axis=2)
    ).astype(ml_dtypes.bfloat16)
    onesp = np.ones((128, 128), dtype=np.float32)
    ident = np.eye(128, dtype=np.float32)

    in_maps = []
    for c in range(NC):
        hcol = slice(c * DH, (c + 1) * DH)

        def pack_w(w):
            return _fp8(
                w[:, hcol].reshape(CT, 128, DH).transpose(1, 0, 2), WS
            )

        in_maps.append(
            {
                "xt8": xt8,
                "wq_c": pack_w(wq),
                "wk_c": pack_w(wk),
                "wv_c": pack_w(wv),
                "xres_c": np.ascontiguousarray(
                    np.concatenate(
                        [
                            xt[:, b * T + c * (TOKC // B):
                               b * T + (c + 1) * (TOKC // B)]
                            for b in range(B)
                        ],
                        axis=1,
                    )
                ),
                "wp8": wp8,
                "w1p": w1p,
                "w2": w2,
                "biaspack": biaspack,
                "b1t": b1t,
                "onesp": onesp,
                "ident": ident,
                "cmask": cmask,
            }
        )
    return in_maps


def _run(inputs, trace=False, debug=False):
    key = "dbg" if debug else "rel"
    if key not in _CACHE:
        _CACHE[key] = _build(debug=debug)
    nc = _CACHE[key]
    in_maps = _pack_inputs(inputs)
    res = bass_utils.run_bass_kernel_spmd(
        nc, in_maps, core_ids=list(range(NC)), trace=trace
    )
    out = np.empty((TOK, C), dtype=np.float32)
    ht = TOKC // B
    for c in range(NC):
        oc = res.results[c]["out"]
        for b in range(B):
            out[b * T + c * ht:b * T + (c + 1) * ht, :] = (
                oc[:, b * ht:(b + 1) * ht].T
            )
    return out.reshape(B, T, C), res


def kernel(**inputs) -> np.ndarray:
    out, _ = _run(inputs, trace=False, debug=False)
    return out
